# revision 8
# baseline (speedup 1.0000x reference)
"""GroupEmbedding Trainium2 kernel (8 NeuronCores, data-parallel over groups).

Computes, per group g:
  out[g] = sum_u sim[g,u] * (user_emb[group_user[g,u]] *
           sum_m counts[g,u,m] * item_emb[behavior_ids[g,u,m]])
  sim[g,u] = 0.5 * dot(similarity_vec[target_user[g]], similarity_vec[group_user[g,u]])

Strategy: shard groups across 8 cores (512 each). On each core, groups are
assigned one-per-partition in waves of 128. All embedding-row gathers are
indirect DMAs (one 256B row per descriptor); reductions over behaviors (M) and
users (U) are free-axis segmented DVE reduces because each partition owns
complete groups.
"""
from contextlib import ExitStack

import numpy as np

import concourse.bass as bass
import concourse.bacc as bacc
import concourse.mybir as mybir
import concourse.tile as tile
from concourse.bass import IndirectOffsetOnAxis
from concourse.bass_utils import run_bass_kernel_spmd

G, U, M = 4096, 50, 20
D = 64
V = 100000
FACTOR = 0.5
NCORES = 8
GPC = G // NCORES            # 512 groups per core
NWAVE = GPC // 128           # 4 waves of 128 groups
UC = 5                       # users per item-gather chunk
CI = UC * M                  # 100 tokens per chunk
NCHUNK = U // UC             # 10 chunks per wave

f32 = mybir.dt.float32
i32 = mybir.dt.int32

_CACHE = {}


def _ensure_ntff_hook():
    """Provide antenv.axon_hooks if the image lacks it, registering the
    ctypes NTFF profile hook against libaxon_pjrt.so (same recipe as
    trn_boot). Needed so run_bass_kernel_spmd(trace=True) can capture HW
    exec time under axon; harmless if tracing is never requested."""
    try:
        import antenv.axon_hooks  # noqa: F401
        return
    except ImportError:
        pass
    import contextlib
    import ctypes
    import sys
    import types

    mod = types.ModuleType("antenv.axon_hooks")
    holder = {}
    mod.set_axon_ntff_profile_hook = lambda h: holder.__setitem__("h", h)
    mod.get_axon_ntff_profile_hook = lambda: holder.get("h")
    try:
        lib = ctypes.CDLL("/opt/axon/libaxon_pjrt.so")
        if hasattr(lib, "axon_start_nrt_profile"):
            lib.axon_start_nrt_profile.argtypes = [
                ctypes.POINTER(ctypes.c_int64), ctypes.c_size_t]
            lib.axon_start_nrt_profile.restype = ctypes.c_int64
            lib.axon_stop_nrt_profile.argtypes = [ctypes.c_char_p]
            lib.axon_stop_nrt_profile.restype = ctypes.c_int64

            @contextlib.contextmanager
            def _hook(output_dir, device_ids):
                import jax
                jax.devices()
                if device_ids:
                    ids = (ctypes.c_int64 * len(device_ids))(*device_ids)
                    rc = lib.axon_start_nrt_profile(ids, len(device_ids))
                else:
                    rc = lib.axon_start_nrt_profile(None, 0)
                if rc != 0:
                    raise RuntimeError(f"axon_start_nrt_profile rc={rc}")
                try:
                    yield
                finally:
                    n = lib.axon_stop_nrt_profile(str(output_dir).encode())
                    print(f"ntff profile: {n} file(s) -> {output_dir}",
                          file=sys.stderr)

            holder["h"] = _hook
    except OSError:
        pass
    import antenv
    sys.modules["antenv.axon_hooks"] = mod
    antenv.axon_hooks = mod


def _build_program():
    nc = bacc.Bacc("TRN2", target_bir_lowering=False, debug=False,
                   num_devices=NCORES)
    item_w = nc.dram_tensor("item_w", [V, D], f32, kind="ExternalInput").ap()
    user_w = nc.dram_tensor("user_w", [V, D], f32, kind="ExternalInput").ap()
    sim_w = nc.dram_tensor("sim_w", [V, D], f32, kind="ExternalInput").ap()
    item_idx = nc.dram_tensor("item_idx", [NWAVE, NCHUNK, 128, CI], i32,
                              kind="ExternalInput").ap()
    counts_r = nc.dram_tensor("counts_r", [NWAVE, NCHUNK, 128, CI], f32,
                              kind="ExternalInput").ap()
    user_idx = nc.dram_tensor("user_idx", [NWAVE, 128, U], i32,
                              kind="ExternalInput").ap()
    targ_idx = nc.dram_tensor("targ_idx", [NWAVE, 128, 1], i32,
                              kind="ExternalInput").ap()
    out = nc.dram_tensor("out", [GPC, D], f32, kind="ExternalOutput").ap()

    with tile.TileContext(nc) as tc:
        with ExitStack() as ctx:
            p_gat = ctx.enter_context(tc.tile_pool(name="gat", bufs=2))
            p_idx = ctx.enter_context(tc.tile_pool(name="idx", bufs=3))
            p_ub = ctx.enter_context(tc.tile_pool(name="ub", bufs=2))
            p_ug = ctx.enter_context(tc.tile_pool(name="ug", bufs=2))
            p_sm = ctx.enter_context(tc.tile_pool(name="sm", bufs=2))

            for w in range(NWAVE):
                ub = p_ub.tile([128, U, D], f32)
                for c in range(NCHUNK):
                    idx_t = p_idx.tile([128, CI], i32, tag="idx")
                    nc.sync.dma_start(idx_t[:], item_idx[w, c])
                    cnt_t = p_idx.tile([128, CI], f32, tag="cnt")
                    nc.sync.dma_start(cnt_t[:], counts_r[w, c])
                    gat = p_gat.tile([128, CI, D], f32)
                    for j in range(CI):
                        nc.gpsimd.indirect_dma_start(
                            out=gat[:, j, :], out_offset=None, in_=item_w[:],
                            in_offset=IndirectOffsetOnAxis(ap=idx_t[:, j:j + 1],
                                                           axis=0))
                    nc.vector.tensor_tensor(
                        out=gat[:], in0=gat[:],
                        in1=cnt_t[:].unsqueeze(2).to_broadcast([128, CI, D]),
                        op=mybir.AluOpType.mult)
                    gat_v = gat[:].rearrange("p (u m) d -> p u d m", u=UC, m=M)
                    nc.vector.reduce_sum(
                        out=ub[:, c * UC:(c + 1) * UC, :], in_=gat_v,
                        axis=mybir.AxisListType.X)
                uidx = p_idx.tile([128, U], i32, tag="uidx")
                nc.sync.dma_start(uidx[:], user_idx[w])
                tidx = p_idx.tile([128, 1], i32, tag="tidx")
                nc.sync.dma_start(tidx[:], targ_idx[w])
                ug = p_ug.tile([128, U, D], f32, tag="ug")
                for j in range(U):
                    nc.gpsimd.indirect_dma_start(
                        out=ug[:, j, :], out_offset=None, in_=user_w[:],
                        in_offset=IndirectOffsetOnAxis(ap=uidx[:, j:j + 1],
                                                       axis=0))
                sg = p_ug.tile([128, U, D], f32, tag="sg")
                for j in range(U):
                    nc.gpsimd.indirect_dma_start(
                        out=sg[:, j, :], out_offset=None, in_=sim_w[:],
                        in_offset=IndirectOffsetOnAxis(ap=uidx[:, j:j + 1],
                                                       axis=0))
                tg = p_sm.tile([128, 1, D], f32, tag="tg")
                nc.gpsimd.indirect_dma_start(
                    out=tg[:, 0, :], out_offset=None, in_=sim_w[:],
                    in_offset=IndirectOffsetOnAxis(ap=tidx[:], axis=0))
                nc.vector.tensor_tensor(out=ub[:], in0=ub[:], in1=ug[:],
                                        op=mybir.AluOpType.mult)
                nc.vector.tensor_tensor(
                    out=sg[:], in0=sg[:], in1=tg[:].to_broadcast([128, U, D]),
                    op=mybir.AluOpType.mult)
                simw = p_sm.tile([128, U], f32, tag="simw")
                nc.vector.reduce_sum(out=simw[:], in_=sg[:],
                                     axis=mybir.AxisListType.X)
                nc.vector.tensor_scalar_mul(out=simw[:], in0=simw[:],
                                            scalar1=FACTOR)
                nc.vector.tensor_tensor(
                    out=ub[:], in0=ub[:],
                    in1=simw[:].unsqueeze(2).to_broadcast([128, U, D]),
                    op=mybir.AluOpType.mult)
                res = p_sm.tile([128, D], f32, tag="res")
                nc.vector.reduce_sum(out=res[:], in_=ub[:].transpose([0, 2, 1]),
                                     axis=mybir.AxisListType.X)
                nc.sync.dma_start(out[w * 128:(w + 1) * 128, :], res[:])
    nc.finalize()
    return nc


def _prep_in_maps(group_user, behavior_ids, behavior_counts, target_user,
                  similarity_vec, user_emb_w, item_emb_w):
    item_w = np.ascontiguousarray(item_emb_w, dtype=np.float32)
    user_w = np.ascontiguousarray(user_emb_w, dtype=np.float32)
    sim_w = np.ascontiguousarray(similarity_vec, dtype=np.float32)

    ii = behavior_ids.reshape(NCORES, NWAVE, 128, NCHUNK, UC, M)
    ii = np.ascontiguousarray(ii.transpose(0, 1, 3, 2, 4, 5)).reshape(
        NCORES, NWAVE, NCHUNK, 128, CI).astype(np.int32, copy=False)
    cc = behavior_counts.reshape(NCORES, NWAVE, 128, NCHUNK, UC, M)
    cc = np.ascontiguousarray(cc.transpose(0, 1, 3, 2, 4, 5)).reshape(
        NCORES, NWAVE, NCHUNK, 128, CI).astype(np.float32, copy=False)
    uu = np.ascontiguousarray(
        group_user.reshape(NCORES, NWAVE, 128, U)).astype(np.int32, copy=False)
    tt = np.ascontiguousarray(
        target_user.reshape(NCORES, NWAVE, 128, 1)).astype(np.int32, copy=False)

    in_maps = []
    for k in range(NCORES):
        in_maps.append({
            "item_w": item_w,
            "user_w": user_w,
            "sim_w": sim_w,
            "item_idx": np.ascontiguousarray(ii[k]),
            "counts_r": np.ascontiguousarray(cc[k]),
            "user_idx": np.ascontiguousarray(uu[k]),
            "targ_idx": np.ascontiguousarray(tt[k]),
        })
    return in_maps


def kernel(group_user, behavior_ids, behavior_counts, target_user,
           similarity_vec, user_emb_w, item_emb_w, _trace=False):
    _ensure_ntff_hook()
    if "nc" not in _CACHE:
        _CACHE["nc"] = _build_program()
    nc = _CACHE["nc"]
    in_maps = _prep_in_maps(group_user, behavior_ids, behavior_counts,
                            target_user, similarity_vec, user_emb_w, item_emb_w)
    r = run_bass_kernel_spmd(nc, in_maps, core_ids=list(range(NCORES)),
                             trace=_trace)
    out = np.concatenate([r.results[k]["out"] for k in range(NCORES)], axis=0)
    _CACHE["last_result"] = r
    return out
